# revision 1
# baseline (speedup 1.0000x reference)
"""LongNet dilated-attention fused Bass/Tile kernel for TRN2 (one core's program).

Math (per core, T tokens of the even-subsampled sequence, model dim D, H heads
of dim 64, NB branches with block sizes blocks[b]):

  for each branch b:
    Q = x W_q^T ; K = x W_k^T ; V = x W_v^T          (+ biases)
    block-diagonal attention with block m_b, softmax over k
    o_b = attn @ V
  y = sum_b o_b W_o[b]^T + biases

Device layout choices (see notes):
  - x is provided TRANSPOSED as xt [D, T] bf16 (host prep).
  - Weights provided transposed: wq [D, 3*D*NB] bf16 (per branch: Q|K|V
    column sections), wo [D*NB, D] bf16.
  - QK GEMM produces Q^T/K^T feature-major [feat, tok]; V GEMM produces V
    token-major [tok, feat] with a ones-column appended per head (65 cols per
    head) so the AV matmul also produces the softmax denominator row.
  - scores^T tiles [k, q] via lhsT=K^T, rhs=Q^T (K=64 contraction; adjacent
    head pairs use partition bases 0/64 -> concurrent PE row groups).
  - exp on ACT (scale folded), pairs of k-chunks share one 2-bank PSUM tile so
    each ACTIVATE covers 2*qsw columns.
  - AV: lhsT = [V_h | ones] [ktok, 65] -> psum [65, q]: rows 0..63 = o^T
    unnormalized, row 64 = denominator s.
  - normalize: rs = 1/s (DVE), broadcast across partitions (GPSIMD
    partition_broadcast), fused into the PSUM->SBUF eviction multiply (DVE).
  - out-projection: lhsT = o^T chunks, rhs = wo tiles, accumulate NB*D/128
    k-chunks in PSUM; eviction adds the (host-folded) output bias row.

Biases: Q/K biases are added exactly during QK eviction (per-partition scalar
add). V bias and output bias are folded on the host into the final bias row
(exact: attn rows sum to 1, so o = A(V + 1 b_v^T) = A V + 1 b_v^T).
"""

import os as _os
import sys as _sys
for _p in ("/opt/trn_rl_repo",):
    if _os.path.isdir(_p) and _p not in _sys.path:
        _sys.path.insert(0, _p)


from contextlib import ExitStack
from dataclasses import dataclass, field

import numpy as np

import concourse.bass as bass
import concourse.mybir as mybir
import concourse.tile as tile
from concourse import library_config

F32 = mybir.dt.float32
BF16 = mybir.dt.bfloat16
P = 128


@dataclass(frozen=True)
class Cfg:
    D: int = 1024
    H: int = 16
    T: int = 2048            # tokens per core
    TG: int = 1024           # token group (attention/out-proj granularity)
    blocks: tuple = (256, 512, 1024)

    @property
    def HD(self):
        return self.D // self.H

    @property
    def NB(self):
        return len(self.blocks)

    @property
    def dc_n(self):
        return self.D // P   # input-feature chunks

    @property
    def nfc(self):
        return self.D // P   # feature chunks per Q (or K) section

    @property
    def scale(self):
        return 1.0 / float(np.sqrt(np.float32(self.HD)))


def build(cfg: Cfg) -> bass.Bass:
    D, H, T, TG = cfg.D, cfg.H, cfg.T, cfg.TG
    NB, dc_n, nfc = cfg.NB, cfg.dc_n, cfg.nfc
    assert cfg.HD == 64, "head pairing assumes head dim 64"
    assert T % TG == 0 and TG % max(cfg.blocks) == 0
    assert min(cfg.blocks) >= P, "k-chunks must start at partition base 0"
    ntk = TG // P            # 128-token chunks per group
    TW = min(512, TG)        # QK gemm token slice
    ndw = min(512, D)        # out-proj N slice
    nnd = D // ndw
    nkc_out = NB * nfc       # out-proj contraction chunks

    nc = bass.Bass(trn_type="TRN2", target_bir_lowering=False)

    xt = nc.dram_tensor("xt", [D, T], BF16, kind="ExternalInput")
    wq = nc.dram_tensor("wq", [D, 3 * D * NB], BF16, kind="ExternalInput")
    wo = nc.dram_tensor("wo", [D * NB, D], BF16, kind="ExternalInput")
    qb = nc.dram_tensor("qb", [P, 3 * nfc * NB], F32, kind="ExternalInput")
    yb = nc.dram_tensor("yb", [1, D], F32, kind="ExternalInput")
    y = nc.dram_tensor("y", [T, D], F32, kind="ExternalOutput")

    kc_max = max(cfg.blocks) // min(P, max(cfg.blocks))  # max k-chunks per block

    with tile.TileContext(nc) as tc, ExitStack() as ctx:
        const = ctx.enter_context(tc.tile_pool(name="const", bufs=1))
        xtp = ctx.enter_context(tc.tile_pool(name="xtp", bufs=1))
        qkp = ctx.enter_context(tc.tile_pool(name="qkp", bufs=1))
        vsp = ctx.enter_context(tc.tile_pool(name="vsp", bufs=1))
        osp = ctx.enter_context(tc.tile_pool(name="osp", bufs=1))
        wqp = ctx.enter_context(tc.tile_pool(name="wqp", bufs=3))
        wvp = ctx.enter_context(tc.tile_pool(name="wvp", bufs=1))
        wop = ctx.enter_context(tc.tile_pool(name="wop", bufs=nkc_out))
        etp = ctx.enter_context(tc.tile_pool(name="etp", bufs=3))
        salp = ctx.enter_context(tc.tile_pool(name="salp", bufs=1))
        stp = ctx.enter_context(tc.tile_pool(name="stp", bufs=2))
        bcp = ctx.enter_context(tc.tile_pool(name="bcp", bufs=3))
        yp = ctx.enter_context(tc.tile_pool(name="yp", bufs=3))
        drp = ctx.enter_context(tc.tile_pool(name="drp", bufs=4, space="DRAM"))
        psS = ctx.enter_context(tc.tile_pool(name="psS", bufs=2, space="PSUM"))
        psA = ctx.enter_context(tc.tile_pool(name="psA", bufs=2, space="PSUM"))
        psG = ctx.enter_context(tc.tile_pool(name="psG", bufs=2, space="PSUM"))

        # ---- constants ----
        qb_s = const.tile([P, 3 * nfc * NB], F32, tag="qb")
        nc.sync.dma_start(qb_s[:], qb.ap())
        # broadcast the bias row across partitions straight from DRAM
        # (0-stride partition read on the DRAM side)
        yb_bc = const.tile([P, D], F32, tag="ybbc")
        nc.sync.dma_start(yb_bc[:], yb.ap()[0, :].partition_broadcast(P))

        for g in range(T // TG):
            tok0 = g * TG
            # x^T slice for this group, [P, dc, TG]
            xtg = xtp.tile([P, dc_n, TG], BF16, tag="xtg")
            nc.sync.dma_start(
                xtg[:], xt.ap()[:, tok0:tok0 + TG].rearrange("(c p) t -> p c t", p=P)
            )
            os_t = osp.tile([P, nkc_out, TG], BF16, tag="os")
            wots0 = []
            for kc in range(nkc_out):
                wt = wop.tile([P, ndw], BF16, tag="wot", name="wt0")
                nc.gpsimd.dma_start(
                    wt[:], wo.ap()[kc * P:(kc + 1) * P, 0:ndw])
                wots0.append(wt)

            for b, m in enumerate(cfg.blocks):
                base = b * 3 * D
                kcw = min(P, m)
                kc_n = m // kcw
                qsw = min(512, m)
                nbl = TG // m

                # ---- QK GEMM (feature-major) ----
                qk = qkp.tile([P, 2 * nfc, TG], BF16, tag="qk")
                for hc in range(nfc):
                    for qki in (0, 1):
                        fcg = qki * nfc + hc
                        wqt = wqp.tile([P, dc_n, P], BF16, tag="wqt")
                        nc.sync.dma_start(
                            wqt[:],
                            wq.ap()[:, base + fcg * P: base + (fcg + 1) * P]
                            .rearrange("(c p) f -> p c f", p=P),
                        )
                        for t2 in range(TG // TW):
                            ps = psG.tile([P, 512], F32, tag="g")
                            for dc in range(dc_n):
                                nc.tensor.matmul(
                                    ps[:, :TW],
                                    wqt[:, dc, :],
                                    xtg[:, dc, t2 * TW:(t2 + 1) * TW],
                                    start=dc == 0,
                                    stop=dc == dc_n - 1,
                                )
                            nc.scalar.add(
                                qk[:, fcg, t2 * TW:(t2 + 1) * TW],
                                ps[:, :TW],
                                qb_s[:, b * 3 * nfc + fcg: b * 3 * nfc + fcg + 1],
                            )

                # ---- V GEMM (token-major, ones column per head) ----
                vs = vsp.tile([P, ntk, H * 65], BF16, tag="vs")
                for h in range(H):
                    nc.gpsimd.memset(vs[:, :, h * 65 + 64:h * 65 + 65], 1.0)
                WV = min(512, D)
                hv = WV // 64           # heads per V slice
                for vf in range(D // WV):
                    wv = wvp.tile([P, dc_n, WV], BF16, tag="wv")
                    nc.sync.dma_start(
                        wv[:],
                        wq.ap()[:, base + 2 * D + vf * WV: base + 2 * D + (vf + 1) * WV]
                        .rearrange("(c p) f -> p c f", p=P),
                    )
                    for tk in range(ntk):
                        ps = psG.tile([P, 512], F32, tag="g")
                        for dc in range(dc_n):
                            nc.tensor.matmul(
                                ps[:, :WV],
                                xtg[:, dc, tk * P:(tk + 1) * P],
                                wv[:, dc, :],
                                start=dc == 0,
                                stop=dc == dc_n - 1,
                            )
                        nc.vector.tensor_copy(
                            vs[:, tk, vf * hv * 65:(vf * hv + hv) * 65]
                            .rearrange("p (h x) -> p h x", x=65)[:, :, 0:64],
                            ps[:, :WV].rearrange("p (h f) -> p h f", f=64),
                        )

                # ---- attention ----
                # denominators for all heads/tokens of this branch-group are
                # collected via base-0 staging rows + SBUF->SBUF DMA into a
                # [H*TG/P, P] layout (row = h*TG/P + q//P, col = q%P) so the
                # reciprocal uses all DVE lanes; engines require 32-aligned
                # start partitions, DMA does not.
                tgp = TG // P
                sal = salp.tile([H * tgp, P], F32, tag="sal")
                salr = salp.tile([H * tgp, P], BF16, tag="salr")
                scr2 = drp.tile([H * tgp, P], BF16, tag="scr2")

                # halves need 32-aligned partition starts for the reciprocal
                can_split = ((H // 2) * tgp) % 32 == 0

                def norm_half(half):
                    """1/s for heads [half*H/2, (half+1)*H/2), bounce, read
                    back replicated, multiply the o^T chunks."""
                    if not can_split:
                        h0, h1 = 0, H
                    else:
                        h0, h1 = half * H // 2, (half + 1) * H // 2
                    r0, r1 = h0 * tgp, h1 * tgp
                    with nc.allow_low_precision(reason="1/s row in bf16"):
                        nc.vector.reciprocal(salr[r0:r1, :], sal[r0:r1, :])
                    nc.gpsimd.dma_start(scr2[r0:r1, :], salr[r0:r1, :])
                    for hp in range(h0 // 2, h1 // 2):
                        bc = bcp.tile([P, TG], BF16, tag="bc")
                        for hh in (0, 1):
                            h = 2 * hp + hh
                            eng = nc.sync if hh == 0 else nc.gpsimd
                            eng.dma_start(
                                bc[hh * 64:(hh + 1) * 64, :]
                                .rearrange("p (a c) -> p a c", c=P),
                                scr2[h * tgp:(h + 1) * tgp, :]
                                .partition_broadcast(64))
                        oc = b * nfc + hp
                        nc.vector.tensor_tensor(
                            os_t[:, oc, :], os_t[:, oc, :], bc[:],
                            mybir.AluOpType.mult)

                for bl in range(nbl):
                    kt0 = bl * m
                    for hp in range(H // 2):
                        fq = hp
                        fk = nfc + hp
                        for qs in range(m // qsw):
                            q0 = kt0 + qs * qsw
                            ets = []
                            for hh in (0, 1):
                                e = etp.tile([P, kc_n, qsw], BF16, tag="et")
                                ets.append(e)
                            # scores^T + exp, k-chunks in pairs sharing a psum tile
                            for kc2 in range((kc_n + 1) // 2):
                                kcs = [kc2 * 2] + ([kc2 * 2 + 1] if kc2 * 2 + 1 < kc_n else [])
                                pss = []
                                for hh in (0, 1):
                                    pss.append(psS.tile([P, 1024], F32, tag="s", name=f"pss{hh}"))
                                for j, kc in enumerate(kcs):
                                    for hh in (0, 1):
                                        nc.tensor.matmul(
                                            pss[hh][:kcw, j * qsw:(j + 1) * qsw],
                                            qk[hh * 64:hh * 64 + 64, fk,
                                               kt0 + kc * kcw: kt0 + (kc + 1) * kcw],
                                            qk[hh * 64:hh * 64 + 64, fq, q0:q0 + qsw],
                                            start=True,
                                            stop=True,
                                        )
                                for hh in (0, 1):
                                    nc.scalar.activation(
                                        ets[hh][:kcw, kc2 * 2: kc2 * 2 + len(kcs), :]
                                        .rearrange("p a b -> p (a b)"),
                                        pss[hh][:kcw, 0:len(kcs) * qsw],
                                        mybir.ActivationFunctionType.Exp,
                                        scale=cfg.scale,
                                    )
                            # AV + denominator, then raw evictions
                            for hh in (0, 1):
                                h = 2 * hp + hh
                                pso = psA.tile([65, 512], F32, tag="a", name="pso")
                                for kc in range(kc_n):
                                    tok = kt0 + kc * kcw
                                    tkc, po = tok // P, tok % P
                                    nc.tensor.matmul(
                                        pso[:, :qsw],
                                        vs[po:po + kcw, tkc, h * 65:(h + 1) * 65],
                                        ets[hh][:kcw, kc, :],
                                        start=kc == 0,
                                        stop=kc == kc_n - 1,
                                    )
                                nc.vector.tensor_copy(
                                    os_t[hh * 64:hh * 64 + 64, b * nfc + hp, q0:q0 + qsw],
                                    pso[0:64, :qsw],
                                )
                                stg = stp.tile([1, 512], F32, tag="stg")
                                nc.vector.tensor_copy(
                                    stg[:, :qsw], pso[64:65, :qsw]
                                )
                                nc.gpsimd.dma_start(
                                    sal[h * tgp + q0 // P:
                                        h * tgp + q0 // P + qsw // P, :],
                                    stg[:1, :qsw],
                                )
                        if can_split and nbl == 1 and hp == H // 4 - 1:
                            norm_half(0)   # heads 0..H/2-1 done; overlap rest
                if not can_split:
                    norm_half(0)
                elif nbl == 1:
                    norm_half(1)
                else:
                    norm_half(0)
                    norm_half(1)

            # ---- out-projection for this group ----
            for nd in range(nnd):
                if nd == 0:
                    wots = wots0
                else:
                    wots = []
                    for kc in range(nkc_out):
                        wt = wop.tile([P, ndw], BF16, tag="wot")
                        nc.gpsimd.dma_start(
                            wt[:], wo.ap()[kc * P:(kc + 1) * P, nd * ndw:(nd + 1) * ndw]
                        )
                        wots.append(wt)
                for tk in range(ntk):
                    psy = psG.tile([P, 512], F32, tag="g")
                    for kc in range(nkc_out):
                        nc.tensor.matmul(
                            psy[:, :ndw],
                            os_t[:, kc, tk * P:(tk + 1) * P],
                            wots[kc][:],
                            start=kc == 0,
                            stop=kc == nkc_out - 1,
                        )
                    ys = yp.tile([P, 512], F32, tag="ys")
                    nc.vector.tensor_tensor(
                        ys[:, :ndw],
                        psy[:, :ndw],
                        yb_bc[:, nd * ndw:(nd + 1) * ndw],
                        mybir.AluOpType.add,
                    )
                    nc.gpsimd.dma_start(
                        y.ap()[tok0 + tk * P: tok0 + (tk + 1) * P, nd * ndw:(nd + 1) * ndw],
                        ys[:, :ndw],
                    )

    return nc


# ---------------- host-side helpers ----------------

def host_prep(cfg: Cfg, weights: dict) -> dict:
    """Build the per-core replicated input tensors from raw nn.Module weights.

    weights: {qkv_w{i}, qkv_b{i}, out_w{i}, out_b{i}} numpy arrays.
    Returns dict of numpy arrays keyed by dram tensor name (minus xt).
    """
    import ml_dtypes

    D, NB, nfc = cfg.D, cfg.NB, cfg.nfc
    bf16 = ml_dtypes.bfloat16
    wq = np.concatenate(
        [np.ascontiguousarray(weights[f"qkv_w{i}"].T) for i in range(NB)], axis=1
    ).astype(bf16)                                   # [D, 3D*NB]
    wo = np.concatenate(
        [np.ascontiguousarray(weights[f"out_w{i}"].T) for i in range(NB)], axis=0
    ).astype(bf16)                                   # [D*NB, D]
    qb = np.zeros((P, 3 * nfc * NB), np.float32)
    for i in range(NB):
        qb[:, i * 3 * nfc:(i + 1) * 3 * nfc] = (
            weights[f"qkv_b{i}"].astype(np.float32).reshape(3 * nfc, P).T
        )
    ybv = np.zeros((D,), np.float64)
    for i in range(NB):
        ybv += weights[f"out_b{i}"].astype(np.float64)
        ybv += weights[f"qkv_b{i}"][2 * D:3 * D].astype(np.float64) @ weights[
            f"out_w{i}"].astype(np.float64).T
    yb = ybv.astype(np.float32).reshape(1, D)
    return {"wq": wq, "wo": wo, "qb": qb, "yb": yb}


# ---------------- harness-facing entry point ----------------
# Shapes hardcoded per the contest contract: x (4, 8192, 1024) fp32, three
# branches of qkv/out weights. All three LongNet branches use rate=2 with
# even segment sizes, so they all read the same even tokens x[:, ::2, :] and
# differ only in attention block size (256/512/1024). The 16384 even tokens
# are split into 8 contiguous shards of 2048 (a multiple of the largest
# block): pure data parallelism, weights replicated, no collectives.

import ml_dtypes
from concourse.bass_utils import run_bass_kernel_spmd

_CFG = Cfg()  # D=1024, H=16, T=2048, TG=1024, blocks=(256, 512, 1024)
N_CORES = 8
B, S = 4, 8192

_NC_CACHE = None


def _split_sync_waits(nc, max_waits=1):
    """This neuronxcc build accepts at most one sync-wait per instruction;
    hoist extras onto their own EventSemaphore instructions (same engine --
    engine waits serialize, so semantics are unchanged)."""
    n = 0
    for f in nc.m.functions:
        for bb in f.blocks:
            out, changed = [], False
            for inst in bb.instructions:
                si = inst.sync_info
                if si is not None and si.on_wait and len(si.on_wait) > max_waits:
                    waits = list(si.on_wait)
                    for w in waits[:-max_waits]:
                        n += 1
                        out.append(mybir.InstEventSemaphore(
                            name=f"I-waitsplit-{n}",
                            engine=inst.engine,
                            sync_info=mybir.SyncInfo(on_wait=[w], on_update=[]),
                        ))
                    inst.sync_info = mybir.SyncInfo(
                        on_wait=waits[-max_waits:], on_update=list(si.on_update))
                    changed = True
                out.append(inst)
            if changed:
                bb.instructions.clear()
                bb.instructions.extend(out)
    return n


def get_nc():
    global _NC_CACHE
    if _NC_CACHE is None:
        nc = build(_CFG)
        _split_sync_waits(nc)
        _NC_CACHE = nc
    return _NC_CACHE


def make_in_maps(inputs):
    x = np.asarray(inputs["x"])
    xe = np.ascontiguousarray(x[:, ::2, :]).reshape(N_CORES, _CFG.T, _CFG.D)
    common = host_prep(_CFG, inputs)
    maps = []
    for c in range(N_CORES):
        mp = dict(common)
        mp["xt"] = np.ascontiguousarray(xe[c].T).astype(ml_dtypes.bfloat16)
        maps.append(mp)
    return maps


def kernel(x, qkv_w0, qkv_b0, out_w0, out_b0,
           qkv_w1, qkv_b1, out_w1, out_b1,
           qkv_w2, qkv_b2, out_w2, out_b2):
    inputs = dict(x=x, qkv_w0=qkv_w0, qkv_b0=qkv_b0, out_w0=out_w0,
                  out_b0=out_b0, qkv_w1=qkv_w1, qkv_b1=qkv_b1, out_w1=out_w1,
                  out_b1=out_b1, qkv_w2=qkv_w2, qkv_b2=qkv_b2, out_w2=out_w2,
                  out_b2=out_b2)
    nc = get_nc()
    in_maps = make_in_maps(inputs)
    res = run_bass_kernel_spmd(nc, in_maps, list(range(N_CORES)))
    yout = np.concatenate([res.results[c]["y"] for c in range(N_CORES)], axis=0)
    return yout.reshape(B, S // 2, _CFG.D)



# revision 2
# speedup vs baseline: 1.1203x; 1.1203x over previous
"""LongNet dilated-attention fused Bass/Tile kernel for TRN2 (one core's program).

Math (per core, T tokens of the even-subsampled sequence, model dim D, H heads
of dim 64, NB branches with block sizes blocks[b]):

  for each branch b:
    Q = x W_q^T ; K = x W_k^T ; V = x W_v^T          (+ biases)
    block-diagonal attention with block m_b, softmax over k
    o_b = attn @ V
  y = sum_b o_b W_o[b]^T + biases

Device layout choices:
  - x is provided TRANSPOSED as xt [D, T] bf16 (host prep).
  - Weights provided transposed: wq [D, 3*D*NB] bf16 (per branch: Q|K|V
    column sections), wo [D*NB, D] bf16.
  - Q^T/K^T are produced per HEAD-PAIR chunk [P, 2, TG] (feature-major), not
    per branch: chunk hp holds Q features [hp*128,(hp+1)*128) on partitions
    (heads 2hp, 2hp+1) and the matching K features. The chunk for head-pair
    hp+1 is emitted as FILLER between the exp-gated attention quanta of
    head-pair hp, so the PE queue never stalls while the ACT engine computes
    exp, and the PE stays busy enough that HAM never throttles it to 1.2 GHz.
  - V GEMM produces V token-major [tok, feat] with a ones-column appended per
    head (65 cols per head) so the AV matmul also produces the softmax
    denominator row.
  - scores^T tiles [k, q] via lhsT=K^T, rhs=Q^T (K=64 contraction; the two
    heads of a pair use partition bases 0/64 -> concurrent PE row groups).
  - exp on ACT (scale folded), pairs of k-chunks share one 2-bank PSUM tile so
    each ACTIVATE covers 2*qsw columns.
  - AV: lhsT = [V_h | ones] [ktok, 65] -> psum [65, q]: rows 0..63 = o^T
    unnormalized, row 64 = denominator s.
  - normalize: rs = 1/s (DVE), broadcast across partitions (DRAM bounce +
    partition-broadcast DMA), fused into a DVE multiply on the o^T buffer.
  - out-projection: lhsT = o^T chunks, rhs = wo tiles, accumulate NB*D/128
    k-chunks in PSUM; eviction adds the (host-folded) output bias row.
    Runs dense at the end of each group (no exp pressure there).

Biases: Q/K biases are added exactly during QK eviction (per-partition scalar
add). V bias and output bias are folded on the host into the final bias row
(exact: attn rows sum to 1, so o = A(V + 1 b_v^T) = A V + 1 b_v^T).
"""

import os as _os
import sys as _sys
for _p in ("/opt/trn_rl_repo",):
    if _os.path.isdir(_p) and _p not in _sys.path:
        _sys.path.insert(0, _p)


from collections import deque
from contextlib import ExitStack
from dataclasses import dataclass

import numpy as np

import concourse.bass as bass
import concourse.mybir as mybir
import concourse.tile as tile

F32 = mybir.dt.float32
BF16 = mybir.dt.bfloat16
P = 128


@dataclass(frozen=True)
class Cfg:
    D: int = 1024
    H: int = 16
    T: int = 2048            # tokens per core
    TG: int = 1024           # token group (attention/out-proj granularity)
    blocks: tuple = (256, 512, 1024)

    @property
    def HD(self):
        return self.D // self.H

    @property
    def NB(self):
        return len(self.blocks)

    @property
    def dc_n(self):
        return self.D // P   # input-feature chunks

    @property
    def nfc(self):
        return self.D // P   # feature chunks per Q (or K) section

    @property
    def scale(self):
        return 1.0 / float(np.sqrt(np.float32(self.HD)))


def build(cfg: Cfg) -> bass.Bass:
    D, H, T, TG = cfg.D, cfg.H, cfg.T, cfg.TG
    NB, dc_n, nfc = cfg.NB, cfg.dc_n, cfg.nfc
    assert cfg.HD == 64, "head pairing assumes head dim 64"
    assert T % TG == 0 and TG % max(cfg.blocks) == 0
    assert min(cfg.blocks) >= P
    ntk = TG // P            # 128-token chunks per group
    tgp = TG // P
    TW = min(512, TG)        # QK gemm token slice
    WV = min(512, D)         # V gemm feature slice
    hv = WV // 64            # heads per V slice
    ndw = min(512, D)        # out-proj N slice
    nnd = D // ndw
    nkc_out = NB * nfc       # out-proj contraction chunks
    ngr = T // TG
    nhp = H // 2             # head pairs == feature chunks per section

    nc = bass.Bass(trn_type="TRN2", target_bir_lowering=False)

    xt = nc.dram_tensor("xt", [D, T], BF16, kind="ExternalInput")
    wq = nc.dram_tensor("wq", [D, 3 * D * NB], BF16, kind="ExternalInput")
    wo = nc.dram_tensor("wo", [D * NB, D], BF16, kind="ExternalInput")
    qb = nc.dram_tensor("qb", [P, 3 * nfc * NB], F32, kind="ExternalInput")
    yb = nc.dram_tensor("yb", [1, D], F32, kind="ExternalInput")
    y = nc.dram_tensor("y", [T, D], F32, kind="ExternalOutput")

    with tile.TileContext(nc) as tc, ExitStack() as ctx:
        const = ctx.enter_context(tc.tile_pool(name="const", bufs=1))
        xtp = ctx.enter_context(tc.tile_pool(name="xtp", bufs=2))
        qkp = ctx.enter_context(tc.tile_pool(name="qkp", bufs=3))
        vsp = ctx.enter_context(tc.tile_pool(name="vsp", bufs=1))
        osp = ctx.enter_context(tc.tile_pool(name="osp", bufs=1))
        wqp = ctx.enter_context(tc.tile_pool(name="wqp", bufs=3))
        wvp = ctx.enter_context(tc.tile_pool(name="wvp", bufs=1))
        wop = ctx.enter_context(tc.tile_pool(name="wop", bufs=nkc_out))
        etp = ctx.enter_context(tc.tile_pool(name="etp", bufs=3))
        salp = ctx.enter_context(tc.tile_pool(name="salp", bufs=1))
        stp = ctx.enter_context(tc.tile_pool(name="stp", bufs=2))
        bcp = ctx.enter_context(tc.tile_pool(name="bcp", bufs=3))
        yp = ctx.enter_context(tc.tile_pool(name="yp", bufs=3))
        drp = ctx.enter_context(tc.tile_pool(name="drp", bufs=4, space="DRAM"))
        psS = ctx.enter_context(tc.tile_pool(name="psS", bufs=2, space="PSUM"))
        psA = ctx.enter_context(tc.tile_pool(name="psA", bufs=2, space="PSUM"))
        psG = ctx.enter_context(tc.tile_pool(name="psG", bufs=2, space="PSUM"))

        # ---- constants ----
        qb_s = const.tile([P, 3 * nfc * NB], F32, tag="qb")
        nc.sync.dma_start(qb_s[:], qb.ap())
        yb_bc = const.tile([P, D], F32, tag="ybbc")
        nc.sync.dma_start(yb_bc[:], yb.ap()[0, :].partition_broadcast(P))

        # V buffer with per-head ones column; the ones columns are written
        # once (V evictions never touch them).
        vs = vsp.tile([P, ntk, H * 65], BF16, tag="vs")
        for h in range(H):
            nc.gpsimd.memset(vs[:, :, h * 65 + 64:h * 65 + 65], 1.0)

        xtg_t = {}

        def load_xtg(g):
            xtg = xtp.tile([P, dc_n, TG], BF16, tag="xtg", name="xtg")
            nc.sync.dma_start(
                xtg[:],
                xt.ap()[:, g * TG:(g + 1) * TG].rearrange("(c p) t -> p c t", p=P),
            )
            xtg_t[g] = xtg

        os_t = {}

        def alloc_os(g):
            os_t[g] = osp.tile([P, nkc_out, TG], BF16, tag="os", name="os")

        # ------------------------------------------------------------------
        # filler machinery: thunks that emit ACT-independent tensor work
        # ------------------------------------------------------------------
        filler = deque()

        def fill(n):
            for _ in range(min(n, len(filler))):
                filler.popleft()()

        def drain():
            fill(len(filler))

        # ---- QK head-pair chunk: qk tile [P, 2(section Q|K), TG] ----
        qk_tiles = {}

        def push_qk_chunk(g, b, hp):
            """Append 16 half-group thunks producing qk chunk (g, b, hp)."""
            base = b * 3 * D
            st = {}

            def ensure():
                if "qk" in st:
                    return
                st["qk"] = qkp.tile([P, 2, TG], BF16, tag="qk", name="qk")
                qk_tiles[(g, b, hp)] = st["qk"]
                for qki in (0, 1):
                    wqt = wqp.tile([P, dc_n, P], BF16, tag="wqt", name="wqt")
                    nc.sync.dma_start(
                        wqt[:],
                        wq.ap()[:, base + qki * D + hp * P:
                                base + qki * D + (hp + 1) * P]
                        .rearrange("(c p) f -> p c f", p=P),
                    )
                    st[f"w{qki}"] = wqt

            def mk(qki, t2, half):
                def th():
                    ensure()
                    xtg = xtg_t[g]
                    wqt = st[f"w{qki}"]
                    if half == 0:
                        st["ps"] = psG.tile([P, 512], F32, tag="g", name="psq")
                    ps = st["ps"]
                    for dc in range(half * (dc_n // 2), (half + 1) * (dc_n // 2)):
                        nc.tensor.matmul(
                            ps[:, :TW],
                            wqt[:, dc, :],
                            xtg[:, dc, t2 * TW:(t2 + 1) * TW],
                            start=dc == 0,
                            stop=dc == dc_n - 1,
                        )
                    if half == 1:
                        col = b * 3 * nfc + qki * nfc + hp
                        nc.scalar.add(
                            st["qk"][:, qki, t2 * TW:(t2 + 1) * TW],
                            ps[:, :TW],
                            qb_s[:, col:col + 1],
                        )
                return th

            for qki in (0, 1):
                for t2 in range(TG // TW):
                    filler.append(mk(qki, t2, 0))
                    filler.append(mk(qki, t2, 1))

        # ---- V gemm slice vf (heads vf*hv .. vf*hv+hv-1) ----
        def push_v_slice(g, b, vf):
            base = b * 3 * D + 2 * D
            st = {}

            def ensure():
                if "wv" in st:
                    return
                wv = wvp.tile([P, dc_n, WV], BF16, tag="wv", name="wv")
                nc.sync.dma_start(
                    wv[:],
                    wq.ap()[:, base + vf * WV: base + (vf + 1) * WV]
                    .rearrange("(c p) f -> p c f", p=P),
                )
                st["wv"] = wv

            def mk(tk, half):
                def th():
                    ensure()
                    xtg = xtg_t[g]
                    if half == 0:
                        st["ps"] = psG.tile([P, 512], F32, tag="g", name="psv")
                    ps = st["ps"]
                    for dc in range(half * (dc_n // 2), (half + 1) * (dc_n // 2)):
                        nc.tensor.matmul(
                            ps[:, :WV],
                            xtg[:, dc, tk * P:(tk + 1) * P],
                            st["wv"][:, dc, :],
                            start=dc == 0,
                            stop=dc == dc_n - 1,
                        )
                    if half == 1:
                        nc.vector.tensor_copy(
                            vs[:, tk, vf * hv * 65:(vf * hv + hv) * 65]
                            .rearrange("p (h x) -> p h x", x=65)[:, :, 0:64],
                            ps[:, :WV].rearrange("p (h f) -> p h f", f=64),
                        )
                return th

            for tk in range(ntk):
                filler.append(mk(tk, 0))
                filler.append(mk(tk, 1))

        # ---- attention unit: one (branch, head-pair, block, q-slice) ----
        def emit_unit(g, b, hp, bl, qs, f_sc, f_av):
            m = cfg.blocks[b]
            kcw = P
            kc_n = m // kcw
            qsw = min(512, m)
            kt0 = bl * m
            q0 = kt0 + qs * qsw
            qk = qk_tiles[(g, b, hp)]
            ets = []
            for hh in (0, 1):
                e = etp.tile([P, kc_n, qsw], BF16, tag="et", name="et")
                ets.append(e)
            for kc2 in range(kc_n // 2):
                kcs = (kc2 * 2, kc2 * 2 + 1)
                pss = []
                for hh in (0, 1):
                    pss.append(psS.tile([P, 1024], F32, tag="s", name=f"pss{hh}"))
                for j, kc in enumerate(kcs):
                    for hh in (0, 1):
                        nc.tensor.matmul(
                            pss[hh][:kcw, j * qsw:(j + 1) * qsw],
                            qk[hh * 64:hh * 64 + 64, 1,
                               kt0 + kc * kcw: kt0 + (kc + 1) * kcw],
                            qk[hh * 64:hh * 64 + 64, 0, q0:q0 + qsw],
                            start=True,
                            stop=True,
                        )
                for hh in (0, 1):
                    nc.scalar.activation(
                        ets[hh][:kcw, kc2 * 2: kc2 * 2 + 2, :]
                        .rearrange("p a b -> p (a b)"),
                        pss[hh][:kcw, 0:2 * qsw],
                        mybir.ActivationFunctionType.Exp,
                        scale=cfg.scale,
                    )
                fill(f_sc)
            # AV + denominator row
            for hh in (0, 1):
                h = 2 * hp + hh
                pso = psA.tile([65, 512], F32, tag="a", name="pso")
                for kc in range(kc_n):
                    tok = kt0 + kc * kcw
                    tkc = tok // P
                    nc.tensor.matmul(
                        pso[:, :qsw],
                        vs[0:kcw, tkc, h * 65:(h + 1) * 65],
                        ets[hh][:kcw, kc, :],
                        start=kc == 0,
                        stop=kc == kc_n - 1,
                    )
                nc.vector.tensor_copy(
                    os_t[g][hh * 64:hh * 64 + 64, b * nfc + hp, q0:q0 + qsw],
                    pso[0:64, :qsw],
                )
                stg = stp.tile([1, 512], F32, tag="stg", name="stg")
                nc.vector.tensor_copy(stg[:, :qsw], pso[64:65, :qsw])
                nc.gpsimd.dma_start(
                    sal_t[h * tgp + q0 // P: h * tgp + q0 // P + qsw // P, :],
                    stg[:1, :qsw],
                )
            fill(f_av)

        # ---- normalization (per branch, per head-half) ----
        def emit_norm_half(g, b, half):
            h0, h1 = half * H // 2, (half + 1) * H // 2
            r0, r1 = h0 * tgp, h1 * tgp
            with nc.allow_low_precision(reason="1/s row in bf16"):
                nc.vector.reciprocal(salr_t[r0:r1, :], sal_t[r0:r1, :])
            nc.gpsimd.dma_start(scr2_t[r0:r1, :], salr_t[r0:r1, :])
            for hp in range(h0 // 2, h1 // 2):
                bc = bcp.tile([P, TG], BF16, tag="bc", name="bc")
                for hh in (0, 1):
                    h = 2 * hp + hh
                    eng = nc.sync if hh == 0 else nc.gpsimd
                    eng.dma_start(
                        bc[hh * 64:(hh + 1) * 64, :]
                        .rearrange("p (a c) -> p a c", c=P),
                        scr2_t[h * tgp:(h + 1) * tgp, :]
                        .partition_broadcast(64))
                oc = b * nfc + hp
                nc.vector.tensor_tensor(
                    os_t[g][:, oc, :], os_t[g][:, oc, :], bc[:],
                    mybir.AluOpType.mult)

        # ---- out-projection for group g (dense) ----
        def emit_out(g, wots0):
            for nd in range(nnd):
                if nd == 0:
                    wots = wots0
                else:
                    wots = []
                    for kc in range(nkc_out):
                        wt = wop.tile([P, ndw], BF16, tag="wot", name="wt")
                        nc.gpsimd.dma_start(
                            wt[:],
                            wo.ap()[kc * P:(kc + 1) * P, nd * ndw:(nd + 1) * ndw],
                        )
                        wots.append(wt)
                for tk in range(ntk):
                    psy = psG.tile([P, 512], F32, tag="g", name="psy")
                    for kc in range(nkc_out):
                        nc.tensor.matmul(
                            psy[:, :ndw],
                            os_t[g][:, kc, tk * P:(tk + 1) * P],
                            wots[kc][:],
                            start=kc == 0,
                            stop=kc == nkc_out - 1,
                        )
                    ys = yp.tile([P, 512], F32, tag="ys", name="ys")
                    nc.vector.tensor_tensor(
                        ys[:, :ndw],
                        psy[:, :ndw],
                        yb_bc[:, nd * ndw:(nd + 1) * ndw],
                        mybir.AluOpType.add,
                    )
                    nc.gpsimd.dma_start(
                        y.ap()[g * TG + tk * P: g * TG + (tk + 1) * P,
                               nd * ndw:(nd + 1) * ndw],
                        ys[:, :ndw],
                    )

        # ------------------------------------------------------------------
        # main schedule
        # ------------------------------------------------------------------
        # filler pops per quantum, tuned so one QK chunk (16 halves) spreads
        # over one head-pair's units: blocks (256,512,1024) have (4,2,2)
        # units/hp and (1,2,4) kc2-quanta per unit.
        F_SC = {0: 2, 1: 2, 2: 2}
        F_AV = {0: 2, 1: 2, 2: 2}

        seq = [(g, b) for g in range(ngr) for b in range(NB)]
        load_xtg(0)
        alloc_os(0)
        push_qk_chunk(0, 0, 0)
        drain()

        for gi, (g, b) in enumerate(seq):
            m = cfg.blocks[b]
            nbl = TG // m
            qsw = min(512, m)
            nqs = m // qsw

            # per-branch denominator staging
            sal_t = salp.tile([H * tgp, P], F32, tag="sal", name="sal")
            salr_t = salp.tile([H * tgp, P], BF16, tag="salr", name="salr")
            scr2_t = drp.tile([H * tgp, P], BF16, tag="scr2", name="scr2")

            drain()
            push_v_slice(g, b, 0)
            drain()

            if b == NB - 1:
                # prefetch for the next group / the out-projection
                if g + 1 < ngr:
                    load_xtg(g + 1)
                wots0 = []
                for kc in range(nkc_out):
                    wt = wop.tile([P, ndw], BF16, tag="wot", name="wt0")
                    nc.gpsimd.dma_start(
                        wt[:], wo.ap()[kc * P:(kc + 1) * P, 0:ndw])
                    wots0.append(wt)

            for hp in range(nhp):
                drain()
                if hp + 1 < nhp:
                    push_qk_chunk(g, b, hp + 1)
                elif gi + 1 < len(seq):
                    g2, b2 = seq[gi + 1]
                    if (g2, b2) != (g, b):
                        if g2 != g:
                            alloc_os(g2)
                        push_qk_chunk(g2, b2, 0)
                if hp == 2 and D > WV:
                    push_v_slice(g, b, 1)
                for bl in range(nbl):
                    for qs in range(nqs):
                        emit_unit(g, b, hp, bl, qs, F_SC[b], F_AV[b])
                if hp == nhp // 2 - 1:
                    emit_norm_half(g, b, 0)
            emit_norm_half(g, b, 1)

            if b == NB - 1:
                drain()
                emit_out(g, wots0)

        drain()

    return nc


# ---------------- host-side helpers ----------------

def host_prep(cfg: Cfg, weights: dict) -> dict:
    """Build the per-core replicated input tensors from raw nn.Module weights.

    weights: {qkv_w{i}, qkv_b{i}, out_w{i}, out_b{i}} numpy arrays.
    Returns dict of numpy arrays keyed by dram tensor name (minus xt).
    """
    import ml_dtypes

    D, NB, nfc = cfg.D, cfg.NB, cfg.nfc
    bf16 = ml_dtypes.bfloat16
    wq = np.concatenate(
        [np.ascontiguousarray(weights[f"qkv_w{i}"].T) for i in range(NB)], axis=1
    ).astype(bf16)                                   # [D, 3D*NB]
    wo = np.concatenate(
        [np.ascontiguousarray(weights[f"out_w{i}"].T) for i in range(NB)], axis=0
    ).astype(bf16)                                   # [D*NB, D]
    qb = np.zeros((P, 3 * nfc * NB), np.float32)
    for i in range(NB):
        qb[:, i * 3 * nfc:(i + 1) * 3 * nfc] = (
            weights[f"qkv_b{i}"].astype(np.float32).reshape(3 * nfc, P).T
        )
    ybv = np.zeros((D,), np.float64)
    for i in range(NB):
        ybv += weights[f"out_b{i}"].astype(np.float64)
        ybv += weights[f"qkv_b{i}"][2 * D:3 * D].astype(np.float64) @ weights[
            f"out_w{i}"].astype(np.float64).T
    yb = ybv.astype(np.float32).reshape(1, D)
    return {"wq": wq, "wo": wo, "qb": qb, "yb": yb}


# ---------------- harness-facing entry point ----------------
# Shapes hardcoded per the contest contract: x (4, 8192, 1024) fp32, three
# branches of qkv/out weights. All three LongNet branches use rate=2 with
# even segment sizes, so they all read the same even tokens x[:, ::2, :] and
# differ only in attention block size (256/512/1024). The 16384 even tokens
# are split into 8 contiguous shards of 2048 (a multiple of the largest
# block): pure data parallelism, weights replicated, no collectives.

import ml_dtypes
from concourse.bass_utils import run_bass_kernel_spmd

_CFG = Cfg()  # D=1024, H=16, T=2048, TG=1024, blocks=(256, 512, 1024)
N_CORES = 8
B, S = 4, 8192

_NC_CACHE = None


def _split_sync_waits(nc, max_waits=1):
    """This neuronxcc build accepts at most one sync-wait per instruction;
    hoist extras onto their own EventSemaphore instructions (same engine --
    engine waits serialize, so semantics are unchanged)."""
    n = 0
    for f in nc.m.functions:
        for bb in f.blocks:
            out, changed = [], False
            for inst in bb.instructions:
                si = inst.sync_info
                if si is not None and si.on_wait and len(si.on_wait) > max_waits:
                    waits = list(si.on_wait)
                    for w in waits[:-max_waits]:
                        n += 1
                        out.append(mybir.InstEventSemaphore(
                            name=f"I-waitsplit-{n}",
                            engine=inst.engine,
                            sync_info=mybir.SyncInfo(on_wait=[w], on_update=[]),
                        ))
                    inst.sync_info = mybir.SyncInfo(
                        on_wait=waits[-max_waits:], on_update=list(si.on_update))
                    changed = True
                out.append(inst)
            if changed:
                bb.instructions.clear()
                bb.instructions.extend(out)
    return n


def get_nc():
    global _NC_CACHE
    if _NC_CACHE is None:
        nc = build(_CFG)
        _split_sync_waits(nc)
        _NC_CACHE = nc
    return _NC_CACHE


def make_in_maps(inputs):
    x = np.asarray(inputs["x"])
    xe = np.ascontiguousarray(x[:, ::2, :]).reshape(N_CORES, _CFG.T, _CFG.D)
    common = host_prep(_CFG, inputs)
    maps = []
    for c in range(N_CORES):
        mp = dict(common)
        mp["xt"] = np.ascontiguousarray(xe[c].T).astype(ml_dtypes.bfloat16)
        maps.append(mp)
    return maps


def kernel(x, qkv_w0, qkv_b0, out_w0, out_b0,
           qkv_w1, qkv_b1, out_w1, out_b1,
           qkv_w2, qkv_b2, out_w2, out_b2):
    inputs = dict(x=x, qkv_w0=qkv_w0, qkv_b0=qkv_b0, out_w0=out_w0,
                  out_b0=out_b0, qkv_w1=qkv_w1, qkv_b1=qkv_b1, out_w1=out_w1,
                  out_b1=out_b1, qkv_w2=qkv_w2, qkv_b2=qkv_b2, out_w2=out_w2,
                  out_b2=out_b2)
    nc = get_nc()
    in_maps = make_in_maps(inputs)
    res = run_bass_kernel_spmd(nc, in_maps, list(range(N_CORES)))
    yout = np.concatenate([res.results[c]["y"] for c in range(N_CORES)], axis=0)
    return yout.reshape(B, S // 2, _CFG.D)


# revision 9
# speedup vs baseline: 1.1540x; 1.0300x over previous
"""LongNet dilated-attention fused Bass/Tile kernel for TRN2 (one core's program).

Math (per core, T tokens of the even-subsampled sequence, model dim D, H heads
of dim 64, NB branches with block sizes blocks[b]):

  for each branch b:
    Q = x W_q^T ; K = x W_k^T ; V = x W_v^T          (+ biases)
    block-diagonal attention with block m_b, softmax over k
    o_b = attn @ V
  y = sum_b o_b W_o[b]^T + biases

Device layout choices:
  - x is provided TRANSPOSED as xt [D, T] bf16 (host prep).
  - Weights provided transposed: wq [D, 3*D*NB] bf16 (per branch: Q|K|V
    column sections), wo [D*NB, D] bf16.
  - Q^T/K^T are produced per HEAD-PAIR chunk [P, 2, TG] (feature-major), not
    per branch: chunk hp holds Q features [hp*128,(hp+1)*128) on partitions
    (heads 2hp, 2hp+1) and the matching K features. The chunk for head-pair
    hp+1 is emitted as FILLER between the exp-gated attention quanta of
    head-pair hp, so the PE queue never stalls while the ACT engine computes
    exp, and the PE stays busy enough that HAM never throttles it to 1.2 GHz.
  - V GEMM produces V token-major [tok, feat] with a ones-column appended per
    head (65 cols per head) so the AV matmul also produces the softmax
    denominator row.
  - scores^T tiles [k, q] via lhsT=K^T, rhs=Q^T (K=64 contraction; the two
    heads of a pair use partition bases 0/64 -> concurrent PE row groups).
  - exp on ACT (scale folded), pairs of k-chunks share one 2-bank PSUM tile so
    each ACTIVATE covers 2*qsw columns.
  - AV: lhsT = [V_h | ones] [ktok, 65] -> psum [65, q]: rows 0..63 = o^T
    unnormalized, row 64 = denominator s.
  - normalize: rs = 1/s (DVE), broadcast across partitions (DRAM bounce +
    partition-broadcast DMA), fused into a DVE multiply on the o^T buffer.
  - out-projection: lhsT = o^T chunks, rhs = wo tiles, accumulate NB*D/128
    k-chunks in PSUM; eviction adds the (host-folded) output bias row.
    Runs dense at the end of each group (no exp pressure there).

Biases: Q/K biases are added exactly during QK eviction (per-partition scalar
add). V bias and output bias are folded on the host into the final bias row
(exact: attn rows sum to 1, so o = A(V + 1 b_v^T) = A V + 1 b_v^T).
"""

import os as _os
import sys as _sys
for _p in ("/opt/trn_rl_repo",):
    if _os.path.isdir(_p) and _p not in _sys.path:
        _sys.path.insert(0, _p)


from collections import deque
from contextlib import ExitStack
from dataclasses import dataclass

import numpy as np

import concourse.bass as bass
import concourse.mybir as mybir
import concourse.tile as tile

F32 = mybir.dt.float32
BF16 = mybir.dt.bfloat16
P = 128


@dataclass(frozen=True)
class Cfg:
    D: int = 1024
    H: int = 16
    T: int = 2048            # tokens per core
    TG: int = 1024           # token group (attention/out-proj granularity)
    blocks: tuple = (256, 512, 1024)

    @property
    def HD(self):
        return self.D // self.H

    @property
    def NB(self):
        return len(self.blocks)

    @property
    def dc_n(self):
        return self.D // P   # input-feature chunks

    @property
    def nfc(self):
        return self.D // P   # feature chunks per Q (or K) section

    @property
    def scale(self):
        return 1.0 / float(np.sqrt(np.float32(self.HD)))


def build(cfg: Cfg) -> bass.Bass:
    D, H, T, TG = cfg.D, cfg.H, cfg.T, cfg.TG
    NB, dc_n, nfc = cfg.NB, cfg.dc_n, cfg.nfc
    assert cfg.HD == 64, "head pairing assumes head dim 64"
    assert T % TG == 0 and TG % max(cfg.blocks) == 0
    assert min(cfg.blocks) >= P
    ntk = TG // P            # 128-token chunks per group
    tgp = TG // P
    TW = min(512, TG)        # QK gemm token slice
    WV = min(512, D)         # V gemm feature slice
    hv = WV // 64            # heads per V slice
    ndw = min(512, D)        # out-proj N slice
    nnd = D // ndw
    nkc_out = NB * nfc       # out-proj contraction chunks
    ngr = T // TG
    nhp = H // 2             # head pairs == feature chunks per section

    nc = bass.Bass(trn_type="TRN2", target_bir_lowering=False)

    xt = nc.dram_tensor("xt", [D, T], BF16, kind="ExternalInput")
    wq = nc.dram_tensor("wq", [D, 3 * D * NB], BF16, kind="ExternalInput")
    wo = nc.dram_tensor("wo", [D * NB, D], BF16, kind="ExternalInput")
    qb = nc.dram_tensor("qb", [P, 3 * nfc * NB], F32, kind="ExternalInput")
    yb = nc.dram_tensor("yb", [1, D], F32, kind="ExternalInput")
    y = nc.dram_tensor("y", [T, D], F32, kind="ExternalOutput")

    with tile.TileContext(nc) as tc, ExitStack() as ctx:
        const = ctx.enter_context(tc.tile_pool(name="const", bufs=1))
        xtp = ctx.enter_context(tc.tile_pool(name="xtp", bufs=2))
        qkp = ctx.enter_context(tc.tile_pool(name="qkp", bufs=3))
        vsp = ctx.enter_context(tc.tile_pool(name="vsp", bufs=1))
        osp = ctx.enter_context(tc.tile_pool(name="osp", bufs=1))
        wqp = ctx.enter_context(tc.tile_pool(name="wqp", bufs=3))
        wvp = ctx.enter_context(tc.tile_pool(name="wvp", bufs=1))
        wop = ctx.enter_context(tc.tile_pool(name="wop", bufs=nkc_out))
        etp = ctx.enter_context(tc.tile_pool(name="etp", bufs=2))
        salp = ctx.enter_context(tc.tile_pool(name="salp", bufs=1))
        stp = ctx.enter_context(tc.tile_pool(name="stp", bufs=2))
        bcp = ctx.enter_context(tc.tile_pool(name="bcp", bufs=3))
        yp = ctx.enter_context(tc.tile_pool(name="yp", bufs=3))
        drp = ctx.enter_context(tc.tile_pool(name="drp", bufs=4, space="DRAM"))
        psS = ctx.enter_context(tc.tile_pool(name="psS", bufs=2, space="PSUM"))
        psA = ctx.enter_context(tc.tile_pool(name="psA", bufs=2, space="PSUM"))
        psG = ctx.enter_context(tc.tile_pool(name="psG", bufs=2, space="PSUM"))

        # ---- constants ----
        qb_s = const.tile([P, 3 * nfc * NB], F32, tag="qb")
        nc.sync.dma_start(qb_s[:], qb.ap())
        yb_bc = const.tile([P, D], F32, tag="ybbc")
        nc.sync.dma_start(yb_bc[:], yb.ap()[0, :].partition_broadcast(P))

        # V buffer with per-head ones column; the ones columns are written
        # once (V evictions never touch them).
        vs = vsp.tile([P, ntk, H * 65], BF16, tag="vs")
        for h in range(H):
            nc.gpsimd.memset(vs[:, :, h * 65 + 64:h * 65 + 65], 1.0)

        xtg_t = {}

        def load_xtg(g):
            xtg = xtp.tile([P, dc_n, TG], BF16, tag="xtg", name="xtg")
            nc.sync.dma_start(
                xtg[:],
                xt.ap()[:, g * TG:(g + 1) * TG].rearrange("(c p) t -> p c t", p=P),
            )
            xtg_t[g] = xtg

        os_t = {}

        def alloc_os(g):
            os_t[g] = osp.tile([P, nkc_out, TG], BF16, tag="os", name="os")

        # ------------------------------------------------------------------
        # filler machinery: thunks that emit ACT-independent tensor work
        # ------------------------------------------------------------------
        filler = deque()

        def fill(n):
            for _ in range(min(n, len(filler))):
                filler.popleft()()

        def drain():
            fill(len(filler))

        # ---- QK head-pair chunk: qk tile [P, 2(section Q|K), TG] ----
        qk_tiles = {}

        def push_qk_chunk(g, b, hp):
            """Append 16 half-group thunks producing qk chunk (g, b, hp)."""
            base = b * 3 * D
            st = {}

            def ensure():
                if "qk" in st:
                    return
                st["qk"] = qkp.tile([P, 2, TG], BF16, tag="qk", name="qk")
                qk_tiles[(g, b, hp)] = st["qk"]
                for qki in (0, 1):
                    wqt = wqp.tile([P, dc_n, P], BF16, tag="wqt", name="wqt")
                    nc.sync.dma_start(
                        wqt[:],
                        wq.ap()[:, base + qki * D + hp * P:
                                base + qki * D + (hp + 1) * P]
                        .rearrange("(c p) f -> p c f", p=P),
                    )
                    st[f"w{qki}"] = wqt

            def mk(qki, t2, half):
                def th():
                    ensure()
                    xtg = xtg_t[g]
                    wqt = st[f"w{qki}"]
                    if half == 0:
                        st["ps"] = psG.tile([P, 512], F32, tag="g", name="psq")
                    ps = st["ps"]
                    for dc in range(half * (dc_n // 2), (half + 1) * (dc_n // 2)):
                        nc.tensor.matmul(
                            ps[:, :TW],
                            wqt[:, dc, :],
                            xtg[:, dc, t2 * TW:(t2 + 1) * TW],
                            start=dc == 0,
                            stop=dc == dc_n - 1,
                        )
                    if half == 1:
                        col = b * 3 * nfc + qki * nfc + hp
                        nc.vector.tensor_scalar_add(
                            st["qk"][:, qki, t2 * TW:(t2 + 1) * TW],
                            ps[:, :TW],
                            qb_s[:, col:col + 1],
                        )
                return th

            for qki in (0, 1):
                for t2 in range(TG // TW):
                    filler.append(mk(qki, t2, 0))
                    filler.append(mk(qki, t2, 1))

        # ---- V gemm slice vf (heads vf*hv .. vf*hv+hv-1) ----
        def push_v_slice(g, b, vf):
            base = b * 3 * D + 2 * D
            st = {}

            def ensure():
                if "wv" in st:
                    return
                wv = wvp.tile([P, dc_n, WV], BF16, tag="wv", name="wv")
                nc.sync.dma_start(
                    wv[:],
                    wq.ap()[:, base + vf * WV: base + (vf + 1) * WV]
                    .rearrange("(c p) f -> p c f", p=P),
                )
                st["wv"] = wv

            def mk(tk, half):
                def th():
                    ensure()
                    xtg = xtg_t[g]
                    if half == 0:
                        st["ps"] = psG.tile([P, 512], F32, tag="g", name="psv")
                    ps = st["ps"]
                    for dc in range(half * (dc_n // 2), (half + 1) * (dc_n // 2)):
                        nc.tensor.matmul(
                            ps[:, :WV],
                            xtg[:, dc, tk * P:(tk + 1) * P],
                            st["wv"][:, dc, :],
                            start=dc == 0,
                            stop=dc == dc_n - 1,
                        )
                    if half == 1:
                        nc.vector.tensor_copy(
                            vs[:, tk, vf * hv * 65:(vf * hv + hv) * 65]
                            .rearrange("p (h x) -> p h x", x=65)[:, :, 0:64],
                            ps[:, :WV].rearrange("p (h f) -> p h f", f=64),
                        )
                return th

            for tk in range(ntk):
                filler.append(mk(tk, 0))
                filler.append(mk(tk, 1))

        # ---- attention unit: one (branch, head-pair, block, q-slice) ----
        # The two heads of a pair share ONE psum tile per k-chunk (head 0 in
        # bank A columns, head 1 in bank B) evicted by ONE exp ACTIVATE, so
        # both scores matmuls wait on the same event and issue concurrently
        # (disjoint PE row groups).
        def emit_unit(g, b, hp, bl, qs, f_sc, f_av):
            m = cfg.blocks[b]
            kcw = P
            kc_n = m // kcw
            qsw = min(512, m)
            kt0 = bl * m
            q0 = kt0 + qs * qsw
            qk = qk_tiles[(g, b, hp)]
            ets = etp.tile([P, 2, kc_n, qsw], BF16, tag="et", name="ets")
            for kc in range(kc_n):
                pss = psS.tile([P, 1024], F32, tag="s", name="pss")
                for hh in (0, 1):
                    nc.tensor.matmul(
                        pss[:kcw, hh * 512: hh * 512 + qsw],
                        qk[hh * 64:hh * 64 + 64, 1,
                           kt0 + kc * kcw: kt0 + (kc + 1) * kcw],
                        qk[hh * 64:hh * 64 + 64, 0, q0:q0 + qsw],
                        start=True,
                        stop=True,
                    )
                nc.scalar.activation(
                    ets[:kcw, :, kc, :],
                    pss[:kcw, :].rearrange("p (a q) -> p a q", a=2)[:, :, :qsw],
                    mybir.ActivationFunctionType.Exp,
                    scale=cfg.scale,
                )
                if kc % 2 == 1:
                    fill(f_sc)
            # AV + denominator row
            for hh in (0, 1):
                h = 2 * hp + hh
                pso = psA.tile([65, 512], F32, tag="a", name="pso")
                for kc in range(kc_n):
                    tok = kt0 + kc * kcw
                    tkc = tok // P
                    nc.tensor.matmul(
                        pso[:, :qsw],
                        vs[0:kcw, tkc, h * 65:(h + 1) * 65],
                        ets[:kcw, hh, kc, :],
                        start=kc == 0,
                        stop=kc == kc_n - 1,
                    )
                nc.vector.tensor_copy(
                    os_t[g][hh * 64:hh * 64 + 64, b * nfc + hp, q0:q0 + qsw],
                    pso[0:64, :qsw],
                )
                stg = stp.tile([1, 512], F32, tag="stg", name="stg")
                nc.vector.tensor_copy(stg[:, :qsw], pso[64:65, :qsw])
                nc.gpsimd.dma_start(
                    sal_t[h * tgp + q0 // P: h * tgp + q0 // P + qsw // P, :],
                    stg[:1, :qsw],
                )
            fill(f_av)

        # ---- normalization (per branch, per head-half) ----
        def emit_norm_half(g, b, half):
            h0, h1 = half * H // 2, (half + 1) * H // 2
            r0, r1 = h0 * tgp, h1 * tgp
            with nc.allow_low_precision(reason="1/s row in bf16"):
                nc.vector.reciprocal(salr_t[r0:r1, :], sal_t[r0:r1, :])
            nc.gpsimd.dma_start(scr2_t[r0:r1, :], salr_t[r0:r1, :])
            for hp in range(h0 // 2, h1 // 2):
                bc = bcp.tile([P, TG], BF16, tag="bc", name="bc")
                for hh in (0, 1):
                    h = 2 * hp + hh
                    eng = nc.sync if hh == 0 else nc.gpsimd
                    eng.dma_start(
                        bc[hh * 64:(hh + 1) * 64, :]
                        .rearrange("p (a c) -> p a c", c=P),
                        scr2_t[h * tgp:(h + 1) * tgp, :]
                        .partition_broadcast(64))
                oc = b * nfc + hp
                nc.vector.tensor_tensor(
                    os_t[g][:, oc, :], os_t[g][:, oc, :], bc[:],
                    mybir.AluOpType.mult)

        # ---- out-projection for group g (dense) ----
        def emit_out(g, wots0):
            for nd in range(nnd):
                if nd == 0:
                    wots = wots0
                else:
                    wots = []
                    for kc in range(nkc_out):
                        wt = wop.tile([P, ndw], BF16, tag="wot", name="wt")
                        nc.gpsimd.dma_start(
                            wt[:],
                            wo.ap()[kc * P:(kc + 1) * P, nd * ndw:(nd + 1) * ndw],
                        )
                        wots.append(wt)
                for tk in range(ntk):
                    psy = psG.tile([P, 512], F32, tag="g", name="psy")
                    for kc in range(nkc_out):
                        nc.tensor.matmul(
                            psy[:, :ndw],
                            os_t[g][:, kc, tk * P:(tk + 1) * P],
                            wots[kc][:],
                            start=kc == 0,
                            stop=kc == nkc_out - 1,
                        )
                    ys = yp.tile([P, 512], F32, tag="ys", name="ys")
                    nc.vector.tensor_tensor(
                        ys[:, :ndw],
                        psy[:, :ndw],
                        yb_bc[:, nd * ndw:(nd + 1) * ndw],
                        mybir.AluOpType.add,
                    )
                    nc.gpsimd.dma_start(
                        y.ap()[g * TG + tk * P: g * TG + (tk + 1) * P,
                               nd * ndw:(nd + 1) * ndw],
                        ys[:, :ndw],
                    )

        # ------------------------------------------------------------------
        # main schedule
        # ------------------------------------------------------------------
        # filler pops per quantum, tuned so one QK chunk (16 halves) spreads
        # over one head-pair's units: blocks (256,512,1024) have (4,2,2)
        # units/hp and (1,2,4) kc2-quanta per unit.
        F_SC = {0: 2, 1: 2, 2: 2}
        F_AV = {0: 2, 1: 2, 2: 2}

        seq = [(g, b) for g in range(ngr) for b in range(NB)]
        load_xtg(0)
        alloc_os(0)
        push_qk_chunk(0, 0, 0)
        drain()

        for gi, (g, b) in enumerate(seq):
            m = cfg.blocks[b]
            nbl = TG // m
            qsw = min(512, m)
            nqs = m // qsw

            # per-branch denominator staging
            sal_t = salp.tile([H * tgp, P], F32, tag="sal", name="sal")
            salr_t = salp.tile([H * tgp, P], BF16, tag="salr", name="salr")
            scr2_t = drp.tile([H * tgp, P], BF16, tag="scr2", name="scr2")

            drain()
            push_v_slice(g, b, 0)
            drain()

            if b == NB - 1:
                # prefetch for the next group / the out-projection
                if g + 1 < ngr:
                    load_xtg(g + 1)
                wots0 = []
                for kc in range(nkc_out):
                    wt = wop.tile([P, ndw], BF16, tag="wot", name="wt0")
                    nc.gpsimd.dma_start(
                        wt[:], wo.ap()[kc * P:(kc + 1) * P, 0:ndw])
                    wots0.append(wt)

            for hp in range(nhp):
                drain()
                if hp + 1 < nhp:
                    push_qk_chunk(g, b, hp + 1)
                elif gi + 1 < len(seq):
                    g2, b2 = seq[gi + 1]
                    if (g2, b2) != (g, b):
                        if g2 != g:
                            alloc_os(g2)
                        push_qk_chunk(g2, b2, 0)
                if hp == 2 and D > WV:
                    push_v_slice(g, b, 1)
                # at the last head-pair before an out-projection, hold back
                # some filler so it can cover the norm -> out-proj latency
                last_pre_out = b == NB - 1 and hp == nhp - 1
                f_sc = 1 if last_pre_out else F_SC[b]
                f_av = 0 if last_pre_out else F_AV[b]
                for bl in range(nbl):
                    for qs in range(nqs):
                        emit_unit(g, b, hp, bl, qs, f_sc, f_av)
                if hp == nhp // 2 - 1:
                    emit_norm_half(g, b, 0)
            emit_norm_half(g, b, 1)

            if b == NB - 1:
                drain()
                emit_out(g, wots0)

        drain()

    return nc


# ---------------- host-side helpers ----------------

def host_prep(cfg: Cfg, weights: dict) -> dict:
    """Build the per-core replicated input tensors from raw nn.Module weights.

    weights: {qkv_w{i}, qkv_b{i}, out_w{i}, out_b{i}} numpy arrays.
    Returns dict of numpy arrays keyed by dram tensor name (minus xt).
    """
    import ml_dtypes

    D, NB, nfc = cfg.D, cfg.NB, cfg.nfc
    bf16 = ml_dtypes.bfloat16
    wq = np.concatenate(
        [np.ascontiguousarray(weights[f"qkv_w{i}"].T) for i in range(NB)], axis=1
    ).astype(bf16)                                   # [D, 3D*NB]
    wo = np.concatenate(
        [np.ascontiguousarray(weights[f"out_w{i}"].T) for i in range(NB)], axis=0
    ).astype(bf16)                                   # [D*NB, D]
    qb = np.zeros((P, 3 * nfc * NB), np.float32)
    for i in range(NB):
        qb[:, i * 3 * nfc:(i + 1) * 3 * nfc] = (
            weights[f"qkv_b{i}"].astype(np.float32).reshape(3 * nfc, P).T
        )
    ybv = np.zeros((D,), np.float64)
    for i in range(NB):
        ybv += weights[f"out_b{i}"].astype(np.float64)
        ybv += weights[f"qkv_b{i}"][2 * D:3 * D].astype(np.float64) @ weights[
            f"out_w{i}"].astype(np.float64).T
    yb = ybv.astype(np.float32).reshape(1, D)
    return {"wq": wq, "wo": wo, "qb": qb, "yb": yb}


# ---------------- harness-facing entry point ----------------
# Shapes hardcoded per the contest contract: x (4, 8192, 1024) fp32, three
# branches of qkv/out weights. All three LongNet branches use rate=2 with
# even segment sizes, so they all read the same even tokens x[:, ::2, :] and
# differ only in attention block size (256/512/1024). The 16384 even tokens
# are split into 8 contiguous shards of 2048 (a multiple of the largest
# block): pure data parallelism, weights replicated, no collectives.

import ml_dtypes
from concourse.bass_utils import run_bass_kernel_spmd

_CFG = Cfg()  # D=1024, H=16, T=2048, TG=1024, blocks=(256, 512, 1024)
N_CORES = 8
B, S = 4, 8192

_NC_CACHE = None


def _split_sync_waits(nc, max_waits=1):
    """This neuronxcc build accepts at most one sync-wait per instruction;
    hoist extras onto their own EventSemaphore instructions (same engine --
    engine waits serialize, so semantics are unchanged)."""
    n = 0
    for f in nc.m.functions:
        for bb in f.blocks:
            out, changed = [], False
            for inst in bb.instructions:
                si = inst.sync_info
                if si is not None and si.on_wait and len(si.on_wait) > max_waits:
                    waits = list(si.on_wait)
                    for w in waits[:-max_waits]:
                        n += 1
                        out.append(mybir.InstEventSemaphore(
                            name=f"I-waitsplit-{n}",
                            engine=inst.engine,
                            sync_info=mybir.SyncInfo(on_wait=[w], on_update=[]),
                        ))
                    inst.sync_info = mybir.SyncInfo(
                        on_wait=waits[-max_waits:], on_update=list(si.on_update))
                    changed = True
                out.append(inst)
            if changed:
                bb.instructions.clear()
                bb.instructions.extend(out)
    return n


def get_nc():
    global _NC_CACHE
    if _NC_CACHE is None:
        nc = build(_CFG)
        _split_sync_waits(nc)
        _NC_CACHE = nc
    return _NC_CACHE


def make_in_maps(inputs):
    x = np.asarray(inputs["x"])
    xe = np.ascontiguousarray(x[:, ::2, :]).reshape(N_CORES, _CFG.T, _CFG.D)
    common = host_prep(_CFG, inputs)
    maps = []
    for c in range(N_CORES):
        mp = dict(common)
        mp["xt"] = np.ascontiguousarray(xe[c].T).astype(ml_dtypes.bfloat16)
        maps.append(mp)
    return maps


def kernel(x, qkv_w0, qkv_b0, out_w0, out_b0,
           qkv_w1, qkv_b1, out_w1, out_b1,
           qkv_w2, qkv_b2, out_w2, out_b2):
    inputs = dict(x=x, qkv_w0=qkv_w0, qkv_b0=qkv_b0, out_w0=out_w0,
                  out_b0=out_b0, qkv_w1=qkv_w1, qkv_b1=qkv_b1, out_w1=out_w1,
                  out_b1=out_b1, qkv_w2=qkv_w2, qkv_b2=qkv_b2, out_w2=out_w2,
                  out_b2=out_b2)
    nc = get_nc()
    in_maps = make_in_maps(inputs)
    res = run_bass_kernel_spmd(nc, in_maps, list(range(N_CORES)))
    yout = np.concatenate([res.results[c]["y"] for c in range(N_CORES)], axis=0)
    return yout.reshape(B, S // 2, _CFG.D)
